# revision 18
# baseline (speedup 1.0000x reference)
"""Conv2d 3x3 (pad=1, stride=1) on 8 TRN2 NeuronCores.

Input  (32, 128, 56, 56) f32, weight (256, 128, 3, 3) f32 -> out (32, 256, 56, 56) f32.

Strategy: data-parallel over batch (4 images per core). Per core, implicit GEMM:
for each of the 9 (kh, kw) taps, a [Cin=128 x Cout=128] stationary matmul against
a shifted window of the zero-padded input streams 448 output pixels (8 rows x 56)
per call, accumulating all 9 taps into one PSUM bank (fp32). Matmul operands are
fp16 (full-rate streaming; fp32 accumulate); the output is stored to HBM as fp16
(Y_DT) and upcast to f32 on the host — total rel err ~3.6e-4.

Measured facts (reps-delta HW timing, 8 cores SPMD):
- Tap-count slope test: 0.456 ns/matmul-row — PE runs at ~full clock (0.417
  ideal; no sustained-load down-throttle), LDWEIGHTS ~90% hidden. PE floor for
  the 504x448-row matmuls/core is ~103 us.
- Fixed (non-PE-scaling) cost was ~65-80 us/rep, dominated by the f32 y store
  (12.9 MB/core). Storing y as fp16 cut it: 201 us -> 154-184 us.
- DMA chunking: partition lines must stay >= ~2.8 KB. Splitting x into 4 and y
  into 4 chunks (1-1.9 KB lines, alternating SP/Act issue) REGRESSED to 218 us.
- tap-outer (weight-stationary) reordering: no gain in cost-model sim (ldweights
  are emitted per-matmul regardless and are already hidden); kept tile-outer.
- fp8 DoubleRow (2x PE rate) is numerically out of reach for randn data
  (~3-5e-2 rel err vs the 2e-2 gate); float32r ~8% slower; float32 4x slower.
"""

import sys

sys.path.insert(0, "/opt/trn_rl_repo")

import numpy as np

import concourse.bass as bass  # noqa: F401
import concourse.mybir as mybir
import concourse.tile as tile
from concourse import bacc
from concourse.bass_utils import run_bass_kernel_spmd

B, CIN, H, W = 32, 128, 56, 56
COUT, KH, KW = 256, 3, 3
NCORES = 8
BPC = B // NCORES  # images per core
HP, WP = H + 2, W + 2  # zero-padded
NPIX = H * W  # 3136
ROWS_PER_TILE = 8
NT = H // ROWS_PER_TILE  # 7 free-dim tiles of 448
NFREE = ROWS_PER_TILE * W  # 448

MM_DT = mybir.dt.float16
F32 = mybir.dt.float32
Y_DT = mybir.dt.float16  # output store dtype: halves y DMA + copy bytes

WARMUP_MMS = 8


def build_conv_bass(reps: int = 1, warmup: int = WARMUP_MMS, xbufs: int = 4,
                    hint: bool = False, mm_dt=None, ydge: str = "scalar",
                    order: str = "tile_outer", y_dt=None, taps: int = KH * KW):
    mm_dt = mm_dt or MM_DT
    y_dt = y_dt or Y_DT
    np_dt = mybir.dt.np(mm_dt)
    nc = bacc.Bacc("TRN2", target_bir_lowering=False, debug=False, num_devices=NCORES)
    x = nc.dram_tensor("x", [BPC, CIN, HP * WP], mm_dt, kind="ExternalInput").ap()
    w = nc.dram_tensor("w", [CIN, KH * KW * COUT], mm_dt, kind="ExternalInput").ap()
    y = nc.dram_tensor("y", [BPC, COUT, NPIX], y_dt, kind="ExternalOutput").ap()

    with tile.TileContext(nc) as tc:
        with (
            tc.tile_pool(name="wp", bufs=1) as wp,
            tc.tile_pool(name="xp", bufs=xbufs) as xp,
            tc.tile_pool(name="op", bufs=2) as op,
            tc.tile_pool(name="pp", bufs=1, space="PSUM") as pp,
        ):
            w_sb = wp.tile([CIN, KH * KW * COUT], mm_dt)
            nc.sync.dma_start(w_sb[:], w)

            if warmup:
                # Warm the PE HAM clock gate while the first input DMA is in
                # flight: memset a scratch tile (no DMA dependency), then spin
                # matmuls on it into a scratch PSUM bank that is never read.
                scratch = wp.tile([128, 640], F32)
                nc.vector.memset(scratch[:], 0.0)
                sc = scratch[:].bitcast(mm_dt)  # >= [128, 640] for any dtype <= 4B
                warm_tag = "ps0" if order == "tap_outer" else "ps"
                warm_bufs = 1 if order == "tap_outer" else 8
                ps_warm = pp.tile([128, 512], F32, name="ps_warm", tag=warm_tag,
                                  bufs=warm_bufs)
                for _ in range(warmup):
                    nc.tensor.matmul(ps_warm[:], sc[:, :128], sc[:, 128:640],
                                     start=True, stop=True)

            # x DMA split: rows [0, 34) cover tiles t=0..3 (rows t*8 .. t*8+9);
            # rows [34, 58) cover t=4..6. Tile tracks sub-tile ranges, so the
            # first matmuls start as soon as the first chunk lands.
            XSPLIT = 34 * WP

            # keep DMA partition lines >= ~2.8KB: finer chunking (1-1.9KB
            # lines) measured 19% SLOWER end-to-end (218us vs 183.6us)
            dma = nc.scalar.dma_start if ydge == "scalar" else nc.sync.dma_start

            def body_tile_outer():
                for n in range(BPC):
                    x_sb = xp.tile([CIN, HP * WP], mm_dt)
                    nc.sync.dma_start(x_sb[:, :XSPLIT], x[n, :, :XSPLIT])
                    nc.sync.dma_start(x_sb[:, XSPLIT:], x[n, :, XSPLIT:])
                    xv = x_sb[:].rearrange("p (h w) -> p h w", h=HP)
                    for m in range(COUT // 128):
                        o_sb = op.tile([128, NPIX], y_dt)
                        for t in range(NT):
                            ps = pp.tile([128, NFREE], F32, name="ps", tag="ps",
                                         bufs=8)
                            for kh in range(KH):
                                for kw in range(KW):
                                    khw = kh * KW + kw
                                    if khw >= taps:
                                        continue
                                    lhsT = w_sb[:, khw * COUT + m * 128 : khw * COUT + m * 128 + 128]
                                    rhs = xv[:, t * ROWS_PER_TILE + kh : t * ROWS_PER_TILE + kh + ROWS_PER_TILE, kw : kw + W]
                                    nc.tensor.matmul(
                                        ps[:], lhsT, rhs,
                                        start=(khw == 0), stop=(khw == taps - 1),
                                    )
                            nc.vector.tensor_copy(o_sb[:, t * NFREE : (t + 1) * NFREE], ps[:])
                        # single store per (n, m): 6.3KB partition lines (vs
                        # 3.6+2.7KB for the 2-chunk split); op bufs=2 hides
                        # the later start behind the next block's compute
                        dma(y[n, m * 128 : (m + 1) * 128, :], o_sb[:])

            def body_tap_outer():
                # Weight-stationary: for each (image, cout block), sweep the 9
                # taps in the outer loop with the 7 row-tiles inner, so each
                # 128x128 stationary weight tile serves 7 back-to-back matmuls
                # (3136 moving rows per LDWEIGHTS instead of 448). All 7 PSUM
                # banks accumulate in place across the tap sweep.
                for n in range(BPC):
                    x_sb = xp.tile([CIN, HP * WP], mm_dt)
                    nc.sync.dma_start(x_sb[:, :XSPLIT], x[n, :, :XSPLIT])
                    nc.sync.dma_start(x_sb[:, XSPLIT:], x[n, :, XSPLIT:])
                    xv = x_sb[:].rearrange("p (h w) -> p h w", h=HP)
                    for m in range(COUT // 128):
                        o_sb = op.tile([128, NPIX], y_dt)
                        ps = [pp.tile([128, NFREE], F32, name=f"ps{t}",
                                      tag=f"ps{t}", bufs=1)
                              for t in range(NT)]
                        for kh in range(KH):
                            for kw in range(KW):
                                khw = kh * KW + kw
                                lhsT = w_sb[:, khw * COUT + m * 128 : khw * COUT + m * 128 + 128]
                                for t in range(NT):
                                    rhs = xv[:, t * ROWS_PER_TILE + kh : t * ROWS_PER_TILE + kh + ROWS_PER_TILE, kw : kw + W]
                                    nc.tensor.matmul(
                                        ps[t][:], lhsT, rhs,
                                        start=(khw == 0), stop=(khw == KH * KW - 1),
                                    )
                                    # drain tile t as soon as its tap sweep ends
                                    if khw == KH * KW - 1:
                                        nc.vector.tensor_copy(
                                            o_sb[:, t * NFREE : (t + 1) * NFREE], ps[t][:])
                                        if t == 3:
                                            dma(y[n, m * 128 : (m + 1) * 128, : 4 * NFREE],
                                                 o_sb[:, : 4 * NFREE])
                        dma(y[n, m * 128 : (m + 1) * 128, 4 * NFREE :],
                             o_sb[:, 4 * NFREE :])

            body = body_tap_outer if order == "tap_outer" else body_tile_outer

            if reps == 1:
                body()
            else:
                with tc.For_i(0, reps, 1,
                              hint_engines=(mybir.EngineType.PE,) if hint else ()):
                    body()
    nc.compile()
    nc._np_mm_dt = np_dt
    nc._np_y_dt = mybir.dt.np(y_dt)
    return nc


_NC_CACHE = None


def _get_nc():
    global _NC_CACHE
    if _NC_CACHE is None:
        _NC_CACHE = build_conv_bass()
    return _NC_CACHE


def run_conv(inputs: np.ndarray, weight: np.ndarray, nc=None, **spmd_kwargs):
    """Returns (output, BassKernelResults)."""
    x = np.ascontiguousarray(np.asarray(inputs, dtype=np.float32))
    w = np.asarray(weight, dtype=np.float32)
    assert x.shape == (B, CIN, H, W) and w.shape == (COUT, CIN, KH, KW)

    nc = nc or _get_nc()
    np_dt = nc._np_mm_dt

    xpad = np.zeros((B, CIN, HP, WP), np_dt)
    xpad[:, :, 1 : H + 1, 1 : W + 1] = x
    xpad = xpad.reshape(B, CIN, HP * WP)
    # w_r[cin, khw*COUT + cout] = weight[cout, cin, kh, kw]
    wr = np.ascontiguousarray(
        w.transpose(1, 2, 3, 0).reshape(CIN, KH * KW * COUT).astype(np_dt)
    )

    in_maps = [
        {"x": xpad[c * BPC : (c + 1) * BPC], "w": wr} for c in range(NCORES)
    ]
    r = run_bass_kernel_spmd(nc, in_maps, core_ids=list(range(NCORES)), **spmd_kwargs)
    out = np.concatenate(
        [r.results[c]["y"].reshape(BPC, COUT, H, W) for c in range(NCORES)], axis=0
    ).astype(np.float32, copy=False)
    return out, r


def kernel(inputs: np.ndarray, weight: np.ndarray) -> np.ndarray:
    out, _ = run_conv(inputs, weight)
    return out



# revision 21
# speedup vs baseline: 1.1188x; 1.1188x over previous
"""Conv2d 3x3 (pad=1, stride=1) on 8 TRN2 NeuronCores.

Input  (32, 128, 56, 56) f32, weight (256, 128, 3, 3) f32 -> out (32, 256, 56, 56) f32.

Strategy: data-parallel over batch (4 images per core). Per core, implicit GEMM:
for each of the 9 (kh, kw) taps, a [Cin=128 x Cout=128] stationary matmul against
a shifted window of the zero-padded input streams 448 output pixels (8 rows x 56)
per call, accumulating all 9 taps into one PSUM bank (fp32). Matmul operands are
fp16 (full-rate streaming; fp32 accumulate); the output is stored to HBM as fp16
(Y_DT) and upcast to f32 on the host — total rel err ~3.6e-4.

Measured facts (reps-delta HW timing, 8 cores SPMD):
- Tap-count slope test: 0.456 ns/matmul-row — PE runs at ~full clock (0.417
  ideal; no sustained-load down-throttle), LDWEIGHTS ~90% hidden. PE floor for
  the 504x448-row matmuls/core is ~103 us.
- Fixed (non-PE-scaling) cost was ~65-80 us/rep, dominated by the f32 y store
  (12.9 MB/core). Storing y as fp16 cut it: 201 us -> 154-184 us.
- DMA chunking: partition lines must stay >= ~2.8 KB. Splitting x into 4 and y
  into 4 chunks (1-1.9 KB lines, alternating SP/Act issue) REGRESSED to 218 us.
- tap-outer (weight-stationary) reordering: no gain in cost-model sim (ldweights
  are emitted per-matmul regardless and are already hidden); kept tile-outer.
- fp8 DoubleRow (2x PE rate) is numerically out of reach for randn data
  (~3-5e-2 rel err vs the 2e-2 gate); float32r ~8% slower; float32 4x slower.
"""

import sys

sys.path.insert(0, "/opt/trn_rl_repo")

import numpy as np

import concourse.bass as bass  # noqa: F401
import concourse.mybir as mybir
import concourse.tile as tile
from concourse import bacc
from concourse.bass_utils import run_bass_kernel_spmd

B, CIN, H, W = 32, 128, 56, 56
COUT, KH, KW = 256, 3, 3
NCORES = 8
BPC = B // NCORES  # images per core
HP, WP = H + 2, W + 2  # zero-padded
NPIX = H * W  # 3136
ROWS_PER_TILE = 8
NT = H // ROWS_PER_TILE  # 7 free-dim tiles of 448
NFREE = ROWS_PER_TILE * W  # 448

MM_DT = mybir.dt.float16
F32 = mybir.dt.float32
Y_DT = mybir.dt.float16  # output store dtype: halves y DMA + copy bytes

WARMUP_MMS = 8


def build_conv_bass(reps: int = 1, warmup: int = WARMUP_MMS, xbufs: int = 4,
                    hint: bool = False, mm_dt=None, ydge: str = "scalar",
                    order: str = "tile_outer", y_dt=None, taps: int = KH * KW):
    mm_dt = mm_dt or MM_DT
    y_dt = y_dt or Y_DT
    np_dt = mybir.dt.np(mm_dt)
    nc = bacc.Bacc("TRN2", target_bir_lowering=False, debug=False, num_devices=NCORES)
    x = nc.dram_tensor("x", [BPC, CIN, HP * WP], mm_dt, kind="ExternalInput").ap()
    w = nc.dram_tensor("w", [CIN, KH * KW * COUT], mm_dt, kind="ExternalInput").ap()
    y = nc.dram_tensor("y", [BPC, COUT, NPIX], y_dt, kind="ExternalOutput").ap()

    with tile.TileContext(nc) as tc:
        with (
            tc.tile_pool(name="wp", bufs=1) as wp,
            tc.tile_pool(name="xp", bufs=xbufs) as xp,
            tc.tile_pool(name="op", bufs=3) as op,
            tc.tile_pool(name="pp", bufs=1, space="PSUM") as pp,
        ):
            w_sb = wp.tile([CIN, KH * KW * COUT], mm_dt)
            nc.sync.dma_start(w_sb[:], w)

            if warmup:
                # Warm the PE HAM clock gate while the first input DMA is in
                # flight: memset a scratch tile (no DMA dependency), then spin
                # matmuls on it into a scratch PSUM bank that is never read.
                scratch = wp.tile([128, 640], F32)
                nc.vector.memset(scratch[:], 0.0)
                sc = scratch[:].bitcast(mm_dt)  # >= [128, 640] for any dtype <= 4B
                warm_tag = "ps0" if order == "tap_outer" else "ps"
                warm_bufs = 1 if order == "tap_outer" else 8
                ps_warm = pp.tile([128, 512], F32, name="ps_warm", tag=warm_tag,
                                  bufs=warm_bufs)
                for _ in range(warmup):
                    nc.tensor.matmul(ps_warm[:], sc[:, :128], sc[:, 128:640],
                                     start=True, stop=True)

            # x DMA split: rows [0, 34) cover tiles t=0..3 (rows t*8 .. t*8+9);
            # rows [34, 58) cover t=4..6. Tile tracks sub-tile ranges, so the
            # first matmuls start as soon as the first chunk lands.
            XSPLIT = 34 * WP

            # keep DMA partition lines >= ~2.8KB: finer chunking (1-1.9KB
            # lines) measured 19% SLOWER end-to-end (218us vs 183.6us)
            dma = nc.scalar.dma_start if ydge == "scalar" else nc.sync.dma_start

            def body_tile_outer():
                for n in range(BPC):
                    x_sb = xp.tile([CIN, HP * WP], mm_dt)
                    nc.sync.dma_start(x_sb[:, :XSPLIT], x[n, :, :XSPLIT])
                    nc.sync.dma_start(x_sb[:, XSPLIT:], x[n, :, XSPLIT:])
                    xv = x_sb[:].rearrange("p (h w) -> p h w", h=HP)
                    for m in range(COUT // 128):
                        o_sb = op.tile([128, NPIX], y_dt)
                        for t in range(NT):
                            ps = pp.tile([128, NFREE], F32, name="ps", tag="ps",
                                         bufs=8)
                            for kh in range(KH):
                                for kw in range(KW):
                                    khw = kh * KW + kw
                                    if khw >= taps:
                                        continue
                                    lhsT = w_sb[:, khw * COUT + m * 128 : khw * COUT + m * 128 + 128]
                                    rhs = xv[:, t * ROWS_PER_TILE + kh : t * ROWS_PER_TILE + kh + ROWS_PER_TILE, kw : kw + W]
                                    nc.tensor.matmul(
                                        ps[:], lhsT, rhs,
                                        start=(khw == 0), stop=(khw == taps - 1),
                                    )
                            # alternate PSUM drains between DVE and Act so
                            # bank reuse isn't gated on one engine's queue
                            if t & 1:
                                nc.scalar.copy(o_sb[:, t * NFREE : (t + 1) * NFREE], ps[:])
                            else:
                                nc.vector.tensor_copy(o_sb[:, t * NFREE : (t + 1) * NFREE], ps[:])
                            if t == 3:
                                dma(
                                    y[n, m * 128 : (m + 1) * 128, : 4 * NFREE],
                                    o_sb[:, : 4 * NFREE],
                                )
                        dma(
                            y[n, m * 128 : (m + 1) * 128, 4 * NFREE :],
                            o_sb[:, 4 * NFREE :],
                        )

            def body_tap_outer():
                # Weight-stationary: for each (image, cout block), sweep the 9
                # taps in the outer loop with the 7 row-tiles inner, so each
                # 128x128 stationary weight tile serves 7 back-to-back matmuls
                # (3136 moving rows per LDWEIGHTS instead of 448). All 7 PSUM
                # banks accumulate in place across the tap sweep.
                for n in range(BPC):
                    x_sb = xp.tile([CIN, HP * WP], mm_dt)
                    nc.sync.dma_start(x_sb[:, :XSPLIT], x[n, :, :XSPLIT])
                    nc.sync.dma_start(x_sb[:, XSPLIT:], x[n, :, XSPLIT:])
                    xv = x_sb[:].rearrange("p (h w) -> p h w", h=HP)
                    for m in range(COUT // 128):
                        o_sb = op.tile([128, NPIX], y_dt)
                        ps = [pp.tile([128, NFREE], F32, name=f"ps{t}",
                                      tag=f"ps{t}", bufs=1)
                              for t in range(NT)]
                        for kh in range(KH):
                            for kw in range(KW):
                                khw = kh * KW + kw
                                lhsT = w_sb[:, khw * COUT + m * 128 : khw * COUT + m * 128 + 128]
                                for t in range(NT):
                                    rhs = xv[:, t * ROWS_PER_TILE + kh : t * ROWS_PER_TILE + kh + ROWS_PER_TILE, kw : kw + W]
                                    nc.tensor.matmul(
                                        ps[t][:], lhsT, rhs,
                                        start=(khw == 0), stop=(khw == KH * KW - 1),
                                    )
                                    # drain tile t as soon as its tap sweep ends
                                    if khw == KH * KW - 1:
                                        nc.vector.tensor_copy(
                                            o_sb[:, t * NFREE : (t + 1) * NFREE], ps[t][:])
                                        if t == 3:
                                            dma(y[n, m * 128 : (m + 1) * 128, : 4 * NFREE],
                                                 o_sb[:, : 4 * NFREE])
                        dma(y[n, m * 128 : (m + 1) * 128, 4 * NFREE :],
                             o_sb[:, 4 * NFREE :])

            body = body_tap_outer if order == "tap_outer" else body_tile_outer

            if reps == 1:
                body()
            else:
                with tc.For_i(0, reps, 1,
                              hint_engines=(mybir.EngineType.PE,) if hint else ()):
                    body()
    nc.compile()
    nc._np_mm_dt = np_dt
    nc._np_y_dt = mybir.dt.np(y_dt)
    return nc


_NC_CACHE = None


def _get_nc():
    global _NC_CACHE
    if _NC_CACHE is None:
        _NC_CACHE = build_conv_bass()
    return _NC_CACHE


def run_conv(inputs: np.ndarray, weight: np.ndarray, nc=None, **spmd_kwargs):
    """Returns (output, BassKernelResults)."""
    x = np.ascontiguousarray(np.asarray(inputs, dtype=np.float32))
    w = np.asarray(weight, dtype=np.float32)
    assert x.shape == (B, CIN, H, W) and w.shape == (COUT, CIN, KH, KW)

    nc = nc or _get_nc()
    np_dt = nc._np_mm_dt

    xpad = np.zeros((B, CIN, HP, WP), np_dt)
    xpad[:, :, 1 : H + 1, 1 : W + 1] = x
    xpad = xpad.reshape(B, CIN, HP * WP)
    # w_r[cin, khw*COUT + cout] = weight[cout, cin, kh, kw]
    wr = np.ascontiguousarray(
        w.transpose(1, 2, 3, 0).reshape(CIN, KH * KW * COUT).astype(np_dt)
    )

    in_maps = [
        {"x": xpad[c * BPC : (c + 1) * BPC], "w": wr} for c in range(NCORES)
    ]
    r = run_bass_kernel_spmd(nc, in_maps, core_ids=list(range(NCORES)), **spmd_kwargs)
    out = np.concatenate(
        [r.results[c]["y"].reshape(BPC, COUT, H, W) for c in range(NCORES)], axis=0
    ).astype(np.float32, copy=False)
    return out, r


def kernel(inputs: np.ndarray, weight: np.ndarray) -> np.ndarray:
    out, _ = run_conv(inputs, weight)
    return out



# revision 23
# speedup vs baseline: 1.1796x; 1.0543x over previous
"""Conv2d 3x3 (pad=1, stride=1) on 8 TRN2 NeuronCores.

Input  (32, 128, 56, 56) f32, weight (256, 128, 3, 3) f32 -> out (32, 256, 56, 56) f32.

Strategy: data-parallel over batch (4 images per core). Per core, implicit GEMM:
for each of the 9 (kh, kw) taps, a [Cin=128 x Cout=128] stationary matmul against
a shifted window of the zero-padded input streams 448 output pixels (8 rows x 56)
per call, accumulating all 9 taps into one PSUM bank (fp32). Matmul operands are
fp16 (full-rate streaming; fp32 accumulate); the output is stored to HBM as fp16
(Y_DT) and upcast to f32 on the host — total rel err ~3.6e-4.

Measured facts (reps-delta HW timing, 8 cores SPMD):
- Tap-count slope test: 0.456 ns/matmul-row — PE runs at ~full clock (0.417
  ideal; no sustained-load down-throttle), LDWEIGHTS ~90% hidden. PE floor for
  the 504x448-row matmuls/core is ~103 us.
- Fixed (non-PE-scaling) cost was ~65-80 us/rep, dominated by the f32 y store
  (12.9 MB/core). Storing y as fp16 cut it: 201 us -> 154-184 us.
- DMA chunking: partition lines must stay >= ~2.8 KB. Splitting x into 4 and y
  into 4 chunks (1-1.9 KB lines, alternating SP/Act issue) REGRESSED to 218 us.
- tap-outer (weight-stationary) reordering: no gain in cost-model sim (ldweights
  are emitted per-matmul regardless and are already hidden); kept tile-outer.
- fp8 DoubleRow (2x PE rate) is numerically out of reach for randn data
  (~3-5e-2 rel err vs the 2e-2 gate); float32r ~8% slower; float32 4x slower.
"""

import sys

sys.path.insert(0, "/opt/trn_rl_repo")

import numpy as np

import concourse.bass as bass  # noqa: F401
import concourse.mybir as mybir
import concourse.tile as tile
from concourse import bacc
from concourse.bass_utils import run_bass_kernel_spmd

B, CIN, H, W = 32, 128, 56, 56
COUT, KH, KW = 256, 3, 3
NCORES = 8
BPC = B // NCORES  # images per core
HP, WP = H + 2, W + 2  # zero-padded
NPIX = H * W  # 3136
ROWS_PER_TILE = 8
NT = H // ROWS_PER_TILE  # 7 free-dim tiles of 448
NFREE = ROWS_PER_TILE * W  # 448

MM_DT = mybir.dt.float16
F32 = mybir.dt.float32
Y_DT = mybir.dt.float16  # output store dtype: halves y DMA + copy bytes

WARMUP_MMS = 8


def build_conv_bass(reps: int = 1, warmup: int = WARMUP_MMS, xbufs: int = 6,
                    hint: bool = False, mm_dt=None, ydge: str = "scalar",
                    order: str = "tile_outer", y_dt=None, taps: int = KH * KW):
    mm_dt = mm_dt or MM_DT
    y_dt = y_dt or Y_DT
    np_dt = mybir.dt.np(mm_dt)
    nc = bacc.Bacc("TRN2", target_bir_lowering=False, debug=False, num_devices=NCORES)
    x = nc.dram_tensor("x", [BPC, CIN, HP * WP], mm_dt, kind="ExternalInput").ap()
    w = nc.dram_tensor("w", [CIN, KH * KW * COUT], mm_dt, kind="ExternalInput").ap()
    y = nc.dram_tensor("y", [BPC, COUT, NPIX], y_dt, kind="ExternalOutput").ap()

    with tile.TileContext(nc) as tc:
        with (
            tc.tile_pool(name="wp", bufs=1) as wp,
            tc.tile_pool(name="xp", bufs=xbufs) as xp,
            tc.tile_pool(name="op", bufs=3) as op,
            tc.tile_pool(name="pp", bufs=1, space="PSUM") as pp,
        ):
            w_sb = wp.tile([CIN, KH * KW * COUT], mm_dt)
            nc.sync.dma_start(w_sb[:], w)

            if warmup:
                # Warm the PE HAM clock gate while the first input DMA is in
                # flight: memset a scratch tile (no DMA dependency), then spin
                # matmuls on it into a scratch PSUM bank that is never read.
                scratch = wp.tile([128, 640], F32)
                nc.vector.memset(scratch[:], 0.0)
                sc = scratch[:].bitcast(mm_dt)  # >= [128, 640] for any dtype <= 4B
                warm_tag = "ps0" if order == "tap_outer" else "ps"
                warm_bufs = 1 if order == "tap_outer" else 8
                ps_warm = pp.tile([128, 512], F32, name="ps_warm", tag=warm_tag,
                                  bufs=warm_bufs)
                for _ in range(warmup):
                    nc.tensor.matmul(ps_warm[:], sc[:, :128], sc[:, 128:640],
                                     start=True, stop=True)

            # x DMA split: rows [0, 34) cover tiles t=0..3 (rows t*8 .. t*8+9);
            # rows [34, 58) cover t=4..6. Tile tracks sub-tile ranges, so the
            # first matmuls start as soon as the first chunk lands.
            XSPLIT = 34 * WP

            # keep DMA partition lines >= ~2.8KB: finer chunking (1-1.9KB
            # lines) measured 19% SLOWER end-to-end (218us vs 183.6us)
            dma = nc.scalar.dma_start if ydge == "scalar" else nc.sync.dma_start

            def body_tile_outer():
                for n in range(BPC):
                    x_sb = xp.tile([CIN, HP * WP], mm_dt)
                    nc.sync.dma_start(x_sb[:, :XSPLIT], x[n, :, :XSPLIT])
                    nc.sync.dma_start(x_sb[:, XSPLIT:], x[n, :, XSPLIT:])
                    xv = x_sb[:].rearrange("p (h w) -> p h w", h=HP)
                    for m in range(COUT // 128):
                        o_sb = op.tile([128, NPIX], y_dt)
                        for t in range(NT):
                            ps = pp.tile([128, NFREE], F32, name="ps", tag="ps",
                                         bufs=8)
                            for kh in range(KH):
                                for kw in range(KW):
                                    khw = kh * KW + kw
                                    if khw >= taps:
                                        continue
                                    lhsT = w_sb[:, khw * COUT + m * 128 : khw * COUT + m * 128 + 128]
                                    rhs = xv[:, t * ROWS_PER_TILE + kh : t * ROWS_PER_TILE + kh + ROWS_PER_TILE, kw : kw + W]
                                    nc.tensor.matmul(
                                        ps[:], lhsT, rhs,
                                        start=(khw == 0), stop=(khw == taps - 1),
                                    )
                            nc.vector.tensor_copy(o_sb[:, t * NFREE : (t + 1) * NFREE], ps[:])
                            if t == 3:
                                dma(
                                    y[n, m * 128 : (m + 1) * 128, : 4 * NFREE],
                                    o_sb[:, : 4 * NFREE],
                                )
                        dma(
                            y[n, m * 128 : (m + 1) * 128, 4 * NFREE :],
                            o_sb[:, 4 * NFREE :],
                        )

            def body_tap_outer():
                # Weight-stationary: for each (image, cout block), sweep the 9
                # taps in the outer loop with the 7 row-tiles inner, so each
                # 128x128 stationary weight tile serves 7 back-to-back matmuls
                # (3136 moving rows per LDWEIGHTS instead of 448). All 7 PSUM
                # banks accumulate in place across the tap sweep.
                for n in range(BPC):
                    x_sb = xp.tile([CIN, HP * WP], mm_dt)
                    nc.sync.dma_start(x_sb[:, :XSPLIT], x[n, :, :XSPLIT])
                    nc.sync.dma_start(x_sb[:, XSPLIT:], x[n, :, XSPLIT:])
                    xv = x_sb[:].rearrange("p (h w) -> p h w", h=HP)
                    for m in range(COUT // 128):
                        o_sb = op.tile([128, NPIX], y_dt)
                        ps = [pp.tile([128, NFREE], F32, name=f"ps{t}",
                                      tag=f"ps{t}", bufs=1)
                              for t in range(NT)]
                        for kh in range(KH):
                            for kw in range(KW):
                                khw = kh * KW + kw
                                lhsT = w_sb[:, khw * COUT + m * 128 : khw * COUT + m * 128 + 128]
                                for t in range(NT):
                                    rhs = xv[:, t * ROWS_PER_TILE + kh : t * ROWS_PER_TILE + kh + ROWS_PER_TILE, kw : kw + W]
                                    nc.tensor.matmul(
                                        ps[t][:], lhsT, rhs,
                                        start=(khw == 0), stop=(khw == KH * KW - 1),
                                    )
                                    # drain tile t as soon as its tap sweep ends
                                    if khw == KH * KW - 1:
                                        nc.vector.tensor_copy(
                                            o_sb[:, t * NFREE : (t + 1) * NFREE], ps[t][:])
                                        if t == 3:
                                            dma(y[n, m * 128 : (m + 1) * 128, : 4 * NFREE],
                                                 o_sb[:, : 4 * NFREE])
                        dma(y[n, m * 128 : (m + 1) * 128, 4 * NFREE :],
                             o_sb[:, 4 * NFREE :])

            body = body_tap_outer if order == "tap_outer" else body_tile_outer

            if reps == 1:
                body()
            else:
                with tc.For_i(0, reps, 1,
                              hint_engines=(mybir.EngineType.PE,) if hint else ()):
                    body()
    nc.compile()
    nc._np_mm_dt = np_dt
    nc._np_y_dt = mybir.dt.np(y_dt)
    return nc


_NC_CACHE = None


def _get_nc():
    global _NC_CACHE
    if _NC_CACHE is None:
        _NC_CACHE = build_conv_bass()
    return _NC_CACHE


def run_conv(inputs: np.ndarray, weight: np.ndarray, nc=None, **spmd_kwargs):
    """Returns (output, BassKernelResults)."""
    x = np.ascontiguousarray(np.asarray(inputs, dtype=np.float32))
    w = np.asarray(weight, dtype=np.float32)
    assert x.shape == (B, CIN, H, W) and w.shape == (COUT, CIN, KH, KW)

    nc = nc or _get_nc()
    np_dt = nc._np_mm_dt

    xpad = np.zeros((B, CIN, HP, WP), np_dt)
    xpad[:, :, 1 : H + 1, 1 : W + 1] = x
    xpad = xpad.reshape(B, CIN, HP * WP)
    # w_r[cin, khw*COUT + cout] = weight[cout, cin, kh, kw]
    wr = np.ascontiguousarray(
        w.transpose(1, 2, 3, 0).reshape(CIN, KH * KW * COUT).astype(np_dt)
    )

    in_maps = [
        {"x": xpad[c * BPC : (c + 1) * BPC], "w": wr} for c in range(NCORES)
    ]
    r = run_bass_kernel_spmd(nc, in_maps, core_ids=list(range(NCORES)), **spmd_kwargs)
    out = np.concatenate(
        [r.results[c]["y"].reshape(BPC, COUT, H, W) for c in range(NCORES)], axis=0
    ).astype(np.float32, copy=False)
    return out, r


def kernel(inputs: np.ndarray, weight: np.ndarray) -> np.ndarray:
    out, _ = run_conv(inputs, weight)
    return out

